# revision 12
# baseline (speedup 1.0000x reference)
"""BitLinear-1.58 (absmean ternary quant + linear) on 8 TRN2 NeuronCores.

Problem: x[4, 2048, 4096] f32, weight[16384, 4096] f32, bias[16384] f32.
    w_q = sign(w) * (|w| >= 0.7 * mean(|w|))   (global mean over all of w)
    y   = x @ w_q.T + bias                      -> [4, 2048, 16384] f32

Sharding (column/tensor parallel): weight & bias sharded along
out_features across 8 cores (2048 each); x replicated. Each core
computes y_shard [8192, 2048]; the host concatenates shards.

Mixed-precision matmul: the 4096-deep contraction is split into
  - k[0:2304)    : bf16 x  (18 k-tiles of 128, regular matmuls)
  - k[2304:4096) : fp8e4m3 x (7 pair-tiles of 256, DoubleRow matmuls:
                   lhsT [128,2,128] loads 2 weights/PE-cell -> 2x MACs)
Ternary wq is exact in both bf16 and fp8; the only extra error is the
e4m3 quantization of x on the fp8 k-range: ~2.65% * sqrt(1792/4096)
~= 1.75%, under the 2e-2 gate.

Per-core device program:
  A: single reversed-order pass over w: 64 [128,1024] f32 slices on 3
     DMA queues, DVE abs-reduce each; stream ends at quant's first
     slices, which stay resident in the 8-buf staging pool (no re-read
     wait after thr). PE ones-matmul partition fold, 8-core AllReduce,
     thr = 0.7 * gsum * 2^-26.
  B: ternary quant wq = (w >= thr) - (w <= -thr), f32 compares (exact
     reference mask semantics), [128,1024] slices alternating between
     DVE and GpSimd; k<2304 stored bf16 [128,2048] k-tiles, k>=2304
     fp8 [128,2,2048] pair-tiles. Re-read of the non-retained 56 slices
     streams on 2 queues behind phase A.
  C: matmul: per (token-tile, oc) one PSUM accumulation group of
     18 bf16 MMs + 7 DoubleRow MMs, + bias, streamed over 64 token
     tiles. First STRIP token tiles run k-synchronously with quant.

x is fed pre-transposed/cast on the host (x^T bf16 [2304, 8192] +
x^T fp8 [1792, 8192], replicated) so the contraction dim lands on
partitions with DMA-friendly layouts.
"""

import numpy as np
import ml_dtypes

import concourse.bacc as bacc
import concourse.mybir as mybir
import concourse.tile as tile
import concourse.bass_utils as bass_utils

F32 = mybir.dt.float32
BF16 = mybir.dt.bfloat16
FP8 = mybir.dt.float8e4
ALU = mybir.AluOpType
AX = mybir.AxisListType
PM = mybir.MatmulPerfMode

N_CORES = 8
B, S, K, O_TOTAL = 4, 2048, 4096, 16384
T = B * S                  # 8192 tokens
O = O_TOTAL // N_CORES     # 2048 out features per core
KT = K // 128              # 32 k-tiles (quant granularity)
KT16 = 18                  # k-tiles computed in bf16
NPT = (KT - KT16) // 2     # 7 fp8 DoubleRow pair-tiles (256 k each)
K16 = KT16 * 128           # 2304
N_OC = O // 512            # 4 output chunks of 512
N_QC = O // 1024           # 2 quant chunks of 1024
NT = T // 128              # 64 token tiles
STRIP = 4                  # leading token tiles, k-synchronous with quant
RWB = 8                    # w staging bufs = retained slices for quant
INV_N = 1.0 / (O_TOTAL * K)  # 2^-26, exact power of two

_NC_CACHE = {}


def build_nc(with_bias: bool):
    nc = bacc.Bacc("TRN2", target_bir_lowering=False, debug=False,
                   num_devices=N_CORES)
    xT = nc.dram_tensor("xT", [K16, T], BF16, kind="ExternalInput")
    xT8 = nc.dram_tensor("xT8", [K - K16, T], FP8, kind="ExternalInput")
    wT = nc.dram_tensor("wT", [K, O], F32, kind="ExternalInput")
    wT16 = nc.dram_tensor("wT16", [K, O], BF16, kind="ExternalInput")
    bias = nc.dram_tensor("bias", [1, O], F32, kind="ExternalInput")
    y = nc.dram_tensor("y", [T, O], F32, kind="ExternalOutput")

    with tile.TileContext(nc) as tc:
        with (
            tc.tile_pool(name="pa", bufs=12) as pa,        # w bf16 phase A
            tc.tile_pool(name="wr", bufs=RWB) as wr,      # w f32 quant feed
            tc.tile_pool(name="mf", bufs=2) as mf,        # quant mask
            tc.tile_pool(name="wq16p", bufs=KT16) as wq16p,
            tc.tile_pool(name="wq8p", bufs=NPT) as wq8p,
            tc.tile_pool(name="xp", bufs=STRIP) as xp,    # x^T bf16 staging
            tc.tile_pool(name="x8p", bufs=STRIP) as x8p,  # x^T fp8 staging
            tc.tile_pool(name="op", bufs=8 if not with_bias else 4) as op,
            tc.tile_pool(name="small", bufs=1) as small,
            tc.tile_pool(name="psum", bufs=8, space="PSUM") as psum,
            tc.tile_pool(name="dram", bufs=1, space="DRAM") as dram,
        ):
            # ---------------- phase A: global absmean threshold ----------
            # Reads the bf16 copy of w (half the bytes of f32; the |w| sum
            # tolerates bf16 quantization: thr shifts ~3e-6 rel, ~74 flips
            # globally, ~0.1% output error). The exact-f32 quant compares
            # read the separate f32 stream below.
            a_engines = [nc.sync, nc.scalar, nc.gpsimd]
            tail_engines = [nc.sync, nc.scalar]
            with nc.named_scope("scaleA"):
                partials = small.tile([128, 2 * KT], F32)
                for i in range(2 * KT):
                    k, half = divmod(i, 2)
                    wt = pa.tile([128, O // 2], BF16, tag="pa",
                                 name=f"wa_{i}")
                    eng = (a_engines[i % 3] if i < 2 * KT - 6
                           else tail_engines[i % 2])
                    eng.dma_start(
                        wt[:], wT16[k * 128:(k + 1) * 128,
                                    half * 1024:(half + 1) * 1024])
                    nc.vector.tensor_reduce(
                        partials[:, i:i + 1], wt[:], AX.X, ALU.add,
                        apply_absolute_value=True)

                col = small.tile([128, 1], F32)
                nc.vector.tensor_reduce(col[:], partials[:], AX.X, ALU.add)
                ones = small.tile([128, 128], F32)
                nc.any.memset(ones[:], 1.0)
                # ones.T @ col = partition-broadcast local sum [128,1]
                ps_scalar = psum.tile([128, 1], F32, tag="acc")
                nc.tensor.matmul(ps_scalar[:], ones[:], col[:])
                local_sum = small.tile([128, 1], F32)
                nc.vector.tensor_copy(local_sum[:], ps_scalar[:])

                in_b = dram.tile([128, 1], F32)
                out_b = dram.tile([128, 1], F32)
                nc.gpsimd.dma_start(in_b[:], local_sum[:])
                nc.gpsimd.collective_compute(
                    "AllReduce", ALU.add,
                    replica_groups=[list(range(N_CORES))],
                    ins=[in_b[:]], outs=[out_b[:]])
                gsum = small.tile([128, 1], F32)
                nc.gpsimd.dma_start(gsum[:], out_b[:])

            # x^T prefetch for the strip tiles: queued behind each
            # queue's phase-A share so it cannot stall the reduce stream.
            xT_r = xT.ap().rearrange("(kt p) t -> p kt t", p=128)
            xT8_r = xT8.ap().rearrange("(pt i p) t -> p pt i t", i=2, p=128)
            x_tiles = {}
            x8_tiles = {}
            for t in range(STRIP):
                x_sb = xp.tile([128, KT16, 128], BF16, tag="x",
                               name=f"x_{t}")
                tail_engines[t % 2].dma_start(
                    x_sb[:], xT_r[:, :, t * 128:(t + 1) * 128])
                x_tiles[t] = x_sb
                x8_sb = x8p.tile([128, NPT, 2, 128], FP8, tag="x8",
                                 name=f"x8_{t}")
                tail_engines[(t + 1) % 2].dma_start(
                    x8_sb[:], xT8_r[:, :, :, t * 128:(t + 1) * 128])
                x8_tiles[t] = x8_sb

            if with_bias:
                bias_sb = small.tile([128, O], F32)
                nc.gpsimd.dma_start(bias_sb[:],
                                    bias.ap().to_broadcast((128, O)))

            # thr = (gsum * 2^-26) * 0.7 ; matches reference rounding
            thr = small.tile([128, 1], F32)
            nc.vector.tensor_scalar(thr[:], gsum[:], INV_N, 0.7,
                                    ALU.mult, ALU.mult)
            nthr = small.tile([128, 1], F32)
            nc.vector.tensor_scalar_mul(nthr[:], thr[:], -1.0)

            # w f32 stream for quant, all 64 slices in consumption
            # order: head on sync/scalar (flows as phase A drains), tail
            # on gpsimd (behind the collective, needed latest).
            wb_tiles = {}
            re_order = [(qc, k) for qc in range(N_QC) for k in range(KT)]
            n_tail = 14
            for j, (qc, k) in enumerate(re_order):
                wt = wr.tile([128, 1024], F32, tag="wr",
                             name=f"wb_{qc}_{k}")
                if j < len(re_order) - n_tail:
                    eng = tail_engines[j % 2]
                else:
                    eng = nc.gpsimd
                eng.dma_start(
                    wt[:], wT[k * 128:(k + 1) * 128,
                              qc * 1024:(qc + 1) * 1024])
                wb_tiles[(qc, k)] = wt

            # quant destination tiles
            wq16 = {}
            for k in range(KT16):
                wq16[k] = wq16p.tile([128, O], BF16, tag="wq16",
                                     name=f"wq16_{k}")
            wq8 = {}
            for pt in range(NPT):
                wq8[pt] = wq8p.tile([128, 2, O], FP8, tag="wq8",
                                    name=f"wq8_{pt}")

            # ---------------- phase B: ternary quant ---------------------
            # wq = (w >= thr) - (w <= -thr); f32 compares; slices alternate
            # between the vector and gpsimd engines.
            def quant_slice(qc, k):
                eng = nc.vector
                wt = wb_tiles[(qc, k)]
                mneg = mf.tile([128, 1024], BF16, tag="mneg",
                               name=f"mneg_{qc}_{k}")
                eng.tensor_scalar(
                    mneg[:], wt[:], nthr[:], None, ALU.is_le)
                if k < KT16:
                    dst = wq16[k][:, qc * 1024:(qc + 1) * 1024]
                else:
                    pt, pl = divmod(k - KT16, 2)
                    dst = wq8[pt][:, pl, qc * 1024:(qc + 1) * 1024]
                eng.scalar_tensor_tensor(
                    dst, wt[:], thr[:], mneg[:], ALU.is_ge, ALU.subtract)

            # ---------------- phase C: matmul + bias ---------------------
            def chain_mms(acc, t, oc):
                o0 = oc * 512
                for k in range(KT16):
                    nc.tensor.matmul(
                        acc[:], x_tiles[t][:, k, :],
                        wq16[k][:, o0:o0 + 512],
                        start=(k == 0), stop=False)
                for pt in range(NPT):
                    nc.tensor.matmul(
                        acc[:], x8_tiles[t][:, pt, :, :],
                        wq8[pt][:, :, o0:o0 + 512],
                        start=False, stop=(pt == NPT - 1),
                        perf_mode=PM.DoubleRow, skip_group_check=True)

            def epilogue(acc, t, oc, ep_engine):
                out_sb = op.tile([128, 512], F32, tag="out",
                                 name=f"o_{t}_{oc}")
                if with_bias:
                    nc.vector.tensor_tensor(
                        out_sb[:], acc[:],
                        bias_sb[:, oc * 512:(oc + 1) * 512], ALU.add)
                elif ep_engine == 0:
                    nc.vector.tensor_copy(out_sb[:], acc[:])
                else:
                    nc.scalar.copy(out_sb[:], acc[:])
                nc.gpsimd.dma_start(
                    y[t * 128:(t + 1) * 128, oc * 512:(oc + 1) * 512],
                    out_sb[:])

            with nc.named_scope("matmulC"):
                # Strip: for each quant chunk qc (o-halves oc=2qc, 2qc+1),
                # run the first STRIP token tiles k-SYNCHRONOUSLY with the
                # quant stream, using all 8 PSUM banks.
                for qc in range(N_QC):
                    accs = {}
                    for t in range(STRIP):
                        for h in range(2):
                            accs[(t, h)] = psum.tile(
                                [128, 512], F32, tag="acc",
                                name=f"sacc_{qc}_{t}_{h}")
                    for k in range(KT):
                        quant_slice(qc, k)
                        if k < KT16:
                            for t in range(STRIP):
                                for h in range(2):
                                    nc.tensor.matmul(
                                        accs[(t, h)][:],
                                        x_tiles[t][:, k, :],
                                        wq16[k][:, qc * 1024 + h * 512:
                                                qc * 1024 + (h + 1) * 512],
                                        start=(k == 0), stop=False)
                        elif (k - KT16) % 2 == 1:
                            pt = (k - KT16) // 2
                            for t in range(STRIP):
                                for h in range(2):
                                    nc.tensor.matmul(
                                        accs[(t, h)][:],
                                        x8_tiles[t][:, pt, :, :],
                                        wq8[pt][:, :, qc * 1024 + h * 512:
                                                qc * 1024 + (h + 1) * 512],
                                        start=False, stop=(k == KT - 1),
                                        perf_mode=PM.DoubleRow,
                                        skip_group_check=True)
                    for t in range(STRIP):
                        for h in range(2):
                            epilogue(accs[(t, h)], t, qc * 2 + h, 1)
                # steady state: token-major
                ep = 0
                for t in range(STRIP, NT):
                    x_sb = xp.tile([128, KT16, 128], BF16, tag="x",
                                   name=f"x_{t}")
                    nc.sync.dma_start(
                        x_sb[:], xT_r[:, :, t * 128:(t + 1) * 128])
                    x_tiles[t] = x_sb
                    x8_sb = x8p.tile([128, NPT, 2, 128], FP8, tag="x8",
                                     name=f"x8_{t}")
                    nc.scalar.dma_start(
                        x8_sb[:], xT8_r[:, :, :, t * 128:(t + 1) * 128])
                    x8_tiles[t] = x8_sb
                    for oc in range(N_OC):
                        acc = psum.tile([128, 512], F32, tag="acc",
                                        name=f"acc_{t}_{oc}")
                        chain_mms(acc, t, oc)
                        epilogue(acc, t, oc, ep)
                        ep ^= 1

    nc.compile()
    return nc


def get_nc(with_bias: bool):
    if with_bias not in _NC_CACHE:
        _NC_CACHE[with_bias] = build_nc(with_bias)
    return _NC_CACHE[with_bias]


def prep_in_maps(x: np.ndarray, weight: np.ndarray, bias: np.ndarray):
    """Host-side sharding/layout: transpose + cast x (replicated; bf16 for
    k<2304, fp8e4m3 for k>=2304), shard weight/bias along out_features."""
    xf = x.reshape(T, K)
    xT = np.ascontiguousarray(xf.T[:K16]).astype(ml_dtypes.bfloat16)
    xT8 = np.ascontiguousarray(xf.T[K16:]).astype(ml_dtypes.float8_e4m3fn)
    wT_full = weight.T  # [K, O_TOTAL] view
    in_maps = []
    for c in range(N_CORES):
        wT_c = np.ascontiguousarray(wT_full[:, c * O:(c + 1) * O])
        in_maps.append({
            "xT": xT,
            "xT8": xT8,
            "wT": wT_c,
            "wT16": wT_c.astype(ml_dtypes.bfloat16),
            "bias": np.ascontiguousarray(
                bias[c * O:(c + 1) * O].reshape(1, O)).astype(np.float32),
        })
    return in_maps


def run_shards(in_maps, trace=False, with_bias=None):
    if with_bias is None:
        with_bias = any(np.any(m["bias"]) for m in in_maps)
    nc = get_nc(with_bias)
    return bass_utils.run_bass_kernel_spmd(
        nc, in_maps, core_ids=list(range(N_CORES)), trace=trace)


def kernel(x: np.ndarray, weight: np.ndarray, bias: np.ndarray) -> np.ndarray:
    x = np.asarray(x, dtype=np.float32)
    weight = np.asarray(weight, dtype=np.float32)
    bias = np.asarray(bias, dtype=np.float32)
    res = run_shards(prep_in_maps(x, weight, bias))
    y = np.concatenate([res.results[c]["y"] for c in range(N_CORES)], axis=1)
    return y.reshape(B, S, O_TOTAL)


# revision 13
# speedup vs baseline: 1.0475x; 1.0475x over previous
"""BitLinear-1.58 (absmean ternary quant + linear) on 8 TRN2 NeuronCores.

Problem: x[4, 2048, 4096] f32, weight[16384, 4096] f32, bias[16384] f32.
    w_q = sign(w) * (|w| >= 0.7 * mean(|w|))   (global mean over all of w)
    y   = x @ w_q.T + bias                      -> [4, 2048, 16384] f32

Sharding (column/tensor parallel): weight & bias sharded along
out_features across 8 cores (2048 each); x replicated. Each core
computes y_shard [8192, 2048]; the host concatenates shards.

Mixed-precision matmul: the 4096-deep contraction is split into
  - k[0:2048)    : bf16 x  (16 k-tiles of 128, regular matmuls)
  - k[2048:4096) : fp8e4m3 x (8 pair-tiles of 256, DoubleRow matmuls:
                   lhsT [128,2,128] loads 2 weights/PE-cell -> 2x MACs)
Ternary wq is exact in both bf16 and fp8; the only extra error is the
e4m3 quantization of x on the fp8 k-range: ~2.65% * sqrt(2048/4096)
~= 1.88%, under the 2e-2 gate.

Per-core device program:
  A: single reversed-order pass over w: 64 [128,1024] f32 slices on 3
     DMA queues, DVE abs-reduce each; stream ends at quant's first
     slices, which stay resident in the 8-buf staging pool (no re-read
     wait after thr). PE ones-matmul partition fold, 8-core AllReduce,
     thr = 0.7 * gsum * 2^-26.
  B: ternary quant wq = (w >= thr) - (w <= -thr), f32 compares (exact
     reference mask semantics), [128,1024] slices alternating between
     DVE and GpSimd; k<2048 stored bf16 [128,2048] k-tiles, k>=2048
     fp8 [128,2,2048] pair-tiles. Re-read of the non-retained 56 slices
     streams on 2 queues behind phase A.
  C: matmul: per (token-tile, oc) one PSUM accumulation group of
     16 bf16 MMs + 8 DoubleRow MMs, + bias, streamed over 64 token
     tiles. First STRIP token tiles run k-synchronously with quant.

x is fed pre-transposed/cast on the host (x^T bf16 [2048, 8192] +
x^T fp8 [2048, 8192], replicated) so the contraction dim lands on
partitions with DMA-friendly layouts.
"""

import numpy as np
import ml_dtypes

import concourse.bacc as bacc
import concourse.mybir as mybir
import concourse.tile as tile
import concourse.bass_utils as bass_utils

F32 = mybir.dt.float32
BF16 = mybir.dt.bfloat16
FP8 = mybir.dt.float8e4
ALU = mybir.AluOpType
AX = mybir.AxisListType
PM = mybir.MatmulPerfMode

N_CORES = 8
B, S, K, O_TOTAL = 4, 2048, 4096, 16384
T = B * S                  # 8192 tokens
O = O_TOTAL // N_CORES     # 2048 out features per core
KT = K // 128              # 32 k-tiles (quant granularity)
KT16 = 16                  # k-tiles computed in bf16
NPT = (KT - KT16) // 2     # 7 fp8 DoubleRow pair-tiles (256 k each)
K16 = KT16 * 128           # 2048
N_OC = O // 512            # 4 output chunks of 512
N_QC = O // 1024           # 2 quant chunks of 1024
NT = T // 128              # 64 token tiles
STRIP = 4                  # leading token tiles, k-synchronous with quant
RWB = 8                    # w staging bufs = retained slices for quant
INV_N = 1.0 / (O_TOTAL * K)  # 2^-26, exact power of two

_NC_CACHE = {}


def build_nc(with_bias: bool):
    nc = bacc.Bacc("TRN2", target_bir_lowering=False, debug=False,
                   num_devices=N_CORES)
    xT = nc.dram_tensor("xT", [K16, T], BF16, kind="ExternalInput")
    xT8 = nc.dram_tensor("xT8", [K - K16, T], FP8, kind="ExternalInput")
    wT = nc.dram_tensor("wT", [K, O], F32, kind="ExternalInput")
    wT16 = nc.dram_tensor("wT16", [K, O], BF16, kind="ExternalInput")
    bias = nc.dram_tensor("bias", [1, O], F32, kind="ExternalInput")
    y = nc.dram_tensor("y", [T, O], F32, kind="ExternalOutput")

    with tile.TileContext(nc) as tc:
        with (
            tc.tile_pool(name="pa", bufs=8) as pa,        # w bf16 phase A
            tc.tile_pool(name="wr", bufs=RWB) as wr,      # w f32 quant feed
            tc.tile_pool(name="mf", bufs=2) as mf,        # quant mask
            tc.tile_pool(name="wq16p", bufs=KT16) as wq16p,
            tc.tile_pool(name="wq8p", bufs=NPT) as wq8p,
            tc.tile_pool(name="xp", bufs=STRIP) as xp,    # x^T bf16 staging
            tc.tile_pool(name="x8p", bufs=STRIP) as x8p,  # x^T fp8 staging
            tc.tile_pool(name="op", bufs=6 if not with_bias else 4) as op,
            tc.tile_pool(name="small", bufs=1) as small,
            tc.tile_pool(name="psum", bufs=8, space="PSUM") as psum,
            tc.tile_pool(name="dram", bufs=1, space="DRAM") as dram,
        ):
            # ---------------- phase A: global absmean threshold ----------
            # Reads the bf16 copy of w (half the bytes of f32; the |w| sum
            # tolerates bf16 quantization: thr shifts ~3e-6 rel, ~74 flips
            # globally, ~0.1% output error). The exact-f32 quant compares
            # read the separate f32 stream below.
            a_engines = [nc.sync, nc.scalar, nc.gpsimd]
            tail_engines = [nc.sync, nc.scalar]
            with nc.named_scope("scaleA"):
                partials = small.tile([128, KT], F32)
                for i in range(KT):
                    wt = pa.tile([128, O], BF16, tag="pa", name=f"wa_{i}")
                    eng = (a_engines[i % 3] if i < KT - 6
                           else tail_engines[i % 2])
                    eng.dma_start(
                        wt[:], wT16[i * 128:(i + 1) * 128, :])
                    nc.vector.tensor_reduce(
                        partials[:, i:i + 1], wt[:], AX.X, ALU.add,
                        apply_absolute_value=True)

                col = small.tile([128, 1], F32)
                nc.vector.tensor_reduce(col[:], partials[:], AX.X, ALU.add)
                ones = small.tile([128, 128], F32)
                nc.any.memset(ones[:], 1.0)
                # ones.T @ col = partition-broadcast local sum [128,1]
                ps_scalar = psum.tile([128, 1], F32, tag="acc")
                nc.tensor.matmul(ps_scalar[:], ones[:], col[:])
                local_sum = small.tile([128, 1], F32)
                nc.vector.tensor_copy(local_sum[:], ps_scalar[:])

                in_b = dram.tile([128, 1], F32)
                out_b = dram.tile([128, 1], F32)
                nc.gpsimd.dma_start(in_b[:], local_sum[:])
                nc.gpsimd.collective_compute(
                    "AllReduce", ALU.add,
                    replica_groups=[list(range(N_CORES))],
                    ins=[in_b[:]], outs=[out_b[:]])
                gsum = small.tile([128, 1], F32)
                nc.gpsimd.dma_start(gsum[:], out_b[:])

            # x^T prefetch for the strip tiles: queued behind each
            # queue's phase-A share so it cannot stall the reduce stream.
            xT_r = xT.ap().rearrange("(kt p) t -> p kt t", p=128)
            xT8_r = xT8.ap().rearrange("(pt i p) t -> p pt i t", i=2, p=128)
            x_tiles = {}
            x8_tiles = {}
            for t in range(STRIP):
                x_sb = xp.tile([128, KT16, 128], BF16, tag="x",
                               name=f"x_{t}")
                tail_engines[t % 2].dma_start(
                    x_sb[:], xT_r[:, :, t * 128:(t + 1) * 128])
                x_tiles[t] = x_sb
                x8_sb = x8p.tile([128, NPT, 2, 128], FP8, tag="x8",
                                 name=f"x8_{t}")
                tail_engines[(t + 1) % 2].dma_start(
                    x8_sb[:], xT8_r[:, :, :, t * 128:(t + 1) * 128])
                x8_tiles[t] = x8_sb

            if with_bias:
                bias_sb = small.tile([128, O], F32)
                nc.gpsimd.dma_start(bias_sb[:],
                                    bias.ap().to_broadcast((128, O)))

            # thr = (gsum * 2^-26) * 0.7 ; matches reference rounding
            thr = small.tile([128, 1], F32)
            nc.vector.tensor_scalar(thr[:], gsum[:], INV_N, 0.7,
                                    ALU.mult, ALU.mult)
            nthr = small.tile([128, 1], F32)
            nc.vector.tensor_scalar_mul(nthr[:], thr[:], -1.0)

            # w f32 stream for quant, all 64 slices in consumption
            # order: head on sync/scalar (flows as phase A drains), tail
            # on gpsimd (behind the collective, needed latest).
            wb_tiles = {}
            re_order = [(qc, k) for qc in range(N_QC) for k in range(KT)]
            n_tail = 14
            for j, (qc, k) in enumerate(re_order):
                wt = wr.tile([128, 1024], F32, tag="wr",
                             name=f"wb_{qc}_{k}")
                if j < len(re_order) - n_tail:
                    eng = tail_engines[j % 2]
                else:
                    eng = nc.gpsimd
                eng.dma_start(
                    wt[:], wT[k * 128:(k + 1) * 128,
                              qc * 1024:(qc + 1) * 1024])
                wb_tiles[(qc, k)] = wt

            # quant destination tiles
            wq16 = {}
            for k in range(KT16):
                wq16[k] = wq16p.tile([128, O], BF16, tag="wq16",
                                     name=f"wq16_{k}")
            wq8 = {}
            for pt in range(NPT):
                wq8[pt] = wq8p.tile([128, 2, O], FP8, tag="wq8",
                                    name=f"wq8_{pt}")

            # ---------------- phase B: ternary quant ---------------------
            # wq = (w >= thr) - (w <= -thr); f32 compares; slices alternate
            # between the vector and gpsimd engines.
            def quant_slice(qc, k):
                eng = nc.vector
                wt = wb_tiles[(qc, k)]
                mneg = mf.tile([128, 1024], BF16, tag="mneg",
                               name=f"mneg_{qc}_{k}")
                eng.tensor_scalar(
                    mneg[:], wt[:], nthr[:], None, ALU.is_le)
                if k < KT16:
                    dst = wq16[k][:, qc * 1024:(qc + 1) * 1024]
                else:
                    pt, pl = divmod(k - KT16, 2)
                    dst = wq8[pt][:, pl, qc * 1024:(qc + 1) * 1024]
                eng.scalar_tensor_tensor(
                    dst, wt[:], thr[:], mneg[:], ALU.is_ge, ALU.subtract)

            # ---------------- phase C: matmul + bias ---------------------
            def chain_mms(acc, t, oc):
                o0 = oc * 512
                for k in range(KT16):
                    nc.tensor.matmul(
                        acc[:], x_tiles[t][:, k, :],
                        wq16[k][:, o0:o0 + 512],
                        start=(k == 0), stop=False)
                for pt in range(NPT):
                    nc.tensor.matmul(
                        acc[:], x8_tiles[t][:, pt, :, :],
                        wq8[pt][:, :, o0:o0 + 512],
                        start=False, stop=(pt == NPT - 1),
                        perf_mode=PM.DoubleRow, skip_group_check=True)

            def epilogue(acc, t, oc, ep_engine):
                out_sb = op.tile([128, 512], F32, tag="out",
                                 name=f"o_{t}_{oc}")
                if with_bias:
                    nc.vector.tensor_tensor(
                        out_sb[:], acc[:],
                        bias_sb[:, oc * 512:(oc + 1) * 512], ALU.add)
                elif ep_engine == 0:
                    nc.vector.tensor_copy(out_sb[:], acc[:])
                else:
                    nc.scalar.copy(out_sb[:], acc[:])
                nc.gpsimd.dma_start(
                    y[t * 128:(t + 1) * 128, oc * 512:(oc + 1) * 512],
                    out_sb[:])

            with nc.named_scope("matmulC"):
                # Strip: for each quant chunk qc (o-halves oc=2qc, 2qc+1),
                # run the first STRIP token tiles k-SYNCHRONOUSLY with the
                # quant stream, using all 8 PSUM banks.
                for qc in range(N_QC):
                    accs = {}
                    for t in range(STRIP):
                        for h in range(2):
                            accs[(t, h)] = psum.tile(
                                [128, 512], F32, tag="acc",
                                name=f"sacc_{qc}_{t}_{h}")
                    for k in range(KT):
                        quant_slice(qc, k)
                        if k < KT16:
                            for t in range(STRIP):
                                for h in range(2):
                                    nc.tensor.matmul(
                                        accs[(t, h)][:],
                                        x_tiles[t][:, k, :],
                                        wq16[k][:, qc * 1024 + h * 512:
                                                qc * 1024 + (h + 1) * 512],
                                        start=(k == 0), stop=False)
                        elif (k - KT16) % 2 == 1:
                            pt = (k - KT16) // 2
                            for t in range(STRIP):
                                for h in range(2):
                                    nc.tensor.matmul(
                                        accs[(t, h)][:],
                                        x8_tiles[t][:, pt, :, :],
                                        wq8[pt][:, :, qc * 1024 + h * 512:
                                                qc * 1024 + (h + 1) * 512],
                                        start=False, stop=(k == KT - 1),
                                        perf_mode=PM.DoubleRow,
                                        skip_group_check=True)
                    for t in range(STRIP):
                        for h in range(2):
                            epilogue(accs[(t, h)], t, qc * 2 + h, 1)
                # steady state: token-major
                ep = 0
                for t in range(STRIP, NT):
                    x_sb = xp.tile([128, KT16, 128], BF16, tag="x",
                                   name=f"x_{t}")
                    nc.sync.dma_start(
                        x_sb[:], xT_r[:, :, t * 128:(t + 1) * 128])
                    x_tiles[t] = x_sb
                    x8_sb = x8p.tile([128, NPT, 2, 128], FP8, tag="x8",
                                     name=f"x8_{t}")
                    nc.scalar.dma_start(
                        x8_sb[:], xT8_r[:, :, :, t * 128:(t + 1) * 128])
                    x8_tiles[t] = x8_sb
                    for oc in range(N_OC):
                        acc = psum.tile([128, 512], F32, tag="acc",
                                        name=f"acc_{t}_{oc}")
                        chain_mms(acc, t, oc)
                        epilogue(acc, t, oc, ep)
                        ep ^= 1

    nc.compile()
    return nc


def get_nc(with_bias: bool):
    if with_bias not in _NC_CACHE:
        _NC_CACHE[with_bias] = build_nc(with_bias)
    return _NC_CACHE[with_bias]


def prep_in_maps(x: np.ndarray, weight: np.ndarray, bias: np.ndarray):
    """Host-side sharding/layout: transpose + cast x (replicated; bf16 for
    k<2048, fp8e4m3 for k>=2048), shard weight/bias along out_features."""
    xf = x.reshape(T, K)
    xT = np.ascontiguousarray(xf.T[:K16]).astype(ml_dtypes.bfloat16)
    xT8 = np.ascontiguousarray(xf.T[K16:]).astype(ml_dtypes.float8_e4m3fn)
    wT_full = weight.T  # [K, O_TOTAL] view
    in_maps = []
    for c in range(N_CORES):
        wT_c = np.ascontiguousarray(wT_full[:, c * O:(c + 1) * O])
        in_maps.append({
            "xT": xT,
            "xT8": xT8,
            "wT": wT_c,
            "wT16": wT_c.astype(ml_dtypes.bfloat16),
            "bias": np.ascontiguousarray(
                bias[c * O:(c + 1) * O].reshape(1, O)).astype(np.float32),
        })
    return in_maps


def run_shards(in_maps, trace=False, with_bias=None):
    if with_bias is None:
        with_bias = any(np.any(m["bias"]) for m in in_maps)
    nc = get_nc(with_bias)
    return bass_utils.run_bass_kernel_spmd(
        nc, in_maps, core_ids=list(range(N_CORES)), trace=trace)


def kernel(x: np.ndarray, weight: np.ndarray, bias: np.ndarray) -> np.ndarray:
    x = np.asarray(x, dtype=np.float32)
    weight = np.asarray(weight, dtype=np.float32)
    bias = np.asarray(bias, dtype=np.float32)
    res = run_shards(prep_in_maps(x, weight, bias))
    y = np.concatenate([res.results[c]["y"] for c in range(N_CORES)], axis=1)
    return y.reshape(B, S, O_TOTAL)
